# revision 6
# baseline (speedup 1.0000x reference)
"""Trainium2 Bass kernel: caching self multi-headed attention (decode step).

Problem: B=32, QLEN=1, DM=1024, H=16, DK=64, TCACHE=4096, fp32 in/out.
  out = MHA(q; KV cache) with QKV projections, cache append, softmax, out-proj.

Sharding (8 NeuronCores): tensor-parallel over heads. Core c owns heads
[2c, 2c+1]: column-parallel wq/wk/wv (128 output dims per core), KV cache
shards on the head dim, row-parallel wo giving a partial [32, 1024] output
per core; the host sums the 8 partials.

v2 design (HW-bound analysis of v1: DMA 88% @ ~314 GB/s AND PE 91% busy with
4096 fp32 LDW+MM pairs; DVE 61% with 1x-rate tensor_reduce):
  * KV cache cast to bf16 on the host -> HBM traffic halves (69 MB/core).
    Host-simulated end-to-end rel err 2.9e-3 (gate 2e-2).
  * All attention math moves to PE as 128-contraction bf16 matmuls:
      scores: per t-chunk j (32 of 128 t), lhsT = K^T chunk [128=(h,d), 128=t]
        (stationary, bf16 fast-weight-load), rhs = Q2 [128, 2] block-column
        (col h holds Q_h on head h's 64 rows, 0 elsewhere)
        -> out [128=t, 2=h] in PSUM, all 32 chunks -> [128, (j,h)] one bank.
      exp: 2 strided ACT ops (one per head) -> e [128,(j,h)] bf16 with
        per-head per-partition denominator accumulation (fp32).
      x:  per t-chunk j, lhsT = V chunk [128=t, 128=(h,d)] (stationary FWL),
        rhs = e[:, j, :] [128, 2] -> out [128=(h,d), 2] accumulated over j in
        PSUM; diagonal (head-matching) half used, off-head column is waste.
    PE cost ~64 small LDW+MM pairs/batch (~2us) -- under the DMA floor.
  * DVE does nothing in the main loop. K rides the sync HWDGE ring, V the
    scalar ring.
  * No per-batch SWDGE q-broadcasts (v1 had 64): Q2 block-columns are built
    once from the projection output with 2 strided copies.

Per-core HBM floor: 69 MB @ ~320-358 GB/s => ~190-215 us.

Softmax skips the max-subtraction: scores ~ N(0,1), exp is safe in fp32 and
mathematically identical to the reference.
"""

import numpy as np
import ml_dtypes
from contextlib import ExitStack

import concourse.bass as bass
import concourse.tile as tile
from concourse import bacc, mybir
from concourse.bass_utils import run_bass_kernel_spmd

F32 = mybir.dt.float32
BF16 = mybir.dt.bfloat16
AX = mybir.AxisListType
ALU = mybir.AluOpType
ACTF = mybir.ActivationFunctionType

B = 32          # batch
DM = 1024       # model dim
H = 16          # total heads
DK = 64         # head dim
T = 4096        # cache length
NCORES = 8
HPC = H // NCORES   # 2 heads per core
HD = HPC * DK       # 128 per-core head dims
NCH = DM // 128     # 8 contraction chunks for the projections
NJ = T // 128       # 32 t-chunks of 128

KV_BUFS = 8         # K/V tile prefetch depth (8 KB/partition each)


def _build_nc(repeat=1, variant="full"):
    # variant: "full" | "dma" (K/V loads only) | "nope" (no V matmuls)
    nc = bacc.Bacc(
        "TRN2",
        target_bir_lowering=False,
        debug=False,
        enable_asserts=False,
        num_devices=NCORES,
    )

    qT8 = nc.dram_tensor("qT8", [128, NCH, B], F32, kind="ExternalInput").ap()
    wq8 = nc.dram_tensor("wq8", [128, NCH, HD], F32, kind="ExternalInput").ap()
    wk8 = nc.dram_tensor("wk8", [128, NCH, HD], F32, kind="ExternalInput").ap()
    wv8 = nc.dram_tensor("wv8", [128, NCH, HD], F32, kind="ExternalInput").ap()
    woT = nc.dram_tensor("woT", [HD, DM], F32, kind="ExternalInput").ap()
    cst = nc.dram_tensor("cst", [128, 11], F32, kind="ExternalInput").ap()
    # K^T: [b, (h,d)=128, t] bf16 ; V: [b, tloc=128, (j, h, d)] bf16
    kc = nc.dram_tensor("kc", [B, HD, T], BF16, kind="ExternalInput").ap()
    vc = nc.dram_tensor("vc", [B, 128, NJ, HPC, DK], BF16, kind="ExternalInput").ap()
    outT = nc.dram_tensor("outT", [128, NCH * B], F32, kind="ExternalOutput").ap()

    vcf = vc.rearrange("b p j h d -> b p (j h d)")

    with ExitStack() as ctx:
        tc = ctx.enter_context(tile.TileContext(nc))
        const = ctx.enter_context(tc.tile_pool(name="const", bufs=1))
        psum = ctx.enter_context(tc.tile_pool(name="psum", bufs=1, space="PSUM"))

        # ---- constants into SBUF ----
        wq_sb = const.tile([128, NCH, HD], F32, tag="wq")
        wk_sb = const.tile([128, NCH, HD], F32, tag="wk")
        wv_sb = const.tile([128, NCH, HD], F32, tag="wv")
        wo_sb = const.tile([HD, DM], F32, tag="wo")
        qT_sb = const.tile([128, NCH, B], F32, tag="qt")
        cst_sb = const.tile([128, 11], F32, tag="cst")
        # weights ride the scalar HWDGE ring so the sync ring starts on the
        # KV stream at t=0 (KV is the critical 69 MB; weights are ~2 MB)
        nc.scalar.dma_start(wq_sb[:], wq8)
        nc.scalar.dma_start(wk_sb[:], wk8)
        nc.scalar.dma_start(wv_sb[:], wv8)
        nc.scalar.dma_start(wo_sb[:], woT)
        nc.scalar.dma_start(qT_sb[:], qT8)
        nc.scalar.dma_start(cst_sb[:], cst)

        ones_sb = const.tile([128, 1], F32, tag="ones")
        onerow_sb = const.tile([1, 64], F32, tag="onerow")
        nc.vector.memset(ones_sb[:], 1.0)
        nc.vector.memset(onerow_sb[:], 1.0)

        dpartA = const.tile([128, B], F32, tag="dpA")  # head-0 denom partials
        dpartB = const.tile([128, B], F32, tag="dpB")  # head-1 denom partials

        # ---- phase 0: projections Q^T, Knew^T, Vnew^T  [128=(h,d), B] ----
        QTp = psum.tile([128, B], F32, tag="p0")
        KTp = psum.tile([128, B], F32, tag="p1")
        VTp = psum.tile([128, B], F32, tag="p2")
        for c in range(NCH):
            st, sp = (c == 0), (c == NCH - 1)
            nc.tensor.matmul(QTp[:], wq_sb[:, c, :], qT_sb[:, c, :], start=st, stop=sp)
        for c in range(NCH):
            st, sp = (c == 0), (c == NCH - 1)
            nc.tensor.matmul(KTp[:], wk_sb[:, c, :], qT_sb[:, c, :], start=st, stop=sp)
        for c in range(NCH):
            st, sp = (c == 0), (c == NCH - 1)
            nc.tensor.matmul(VTp[:], wv_sb[:, c, :], qT_sb[:, c, :], start=st, stop=sp)

        QT_sb = const.tile([128, B], F32, tag="QT")
        KnT_sb = const.tile([128, B], F32, tag="KnT")
        VnT_sb = const.tile([128, B], F32, tag="VnT")
        nc.scalar.activation(QT_sb[:], QTp[:], ACTF.Identity, bias=cst_sb[:, 0:1], scale=1.0)
        nc.scalar.activation(KnT_sb[:], KTp[:], ACTF.Identity, bias=cst_sb[:, 1:2], scale=1.0)
        nc.scalar.activation(VnT_sb[:], VTp[:], ACTF.Identity, bias=cst_sb[:, 2:3], scale=1.0)

        # Q2 block-columns [128, B, 2] bf16: col (b,h) = Q_h masked to head h rows
        Q2_sb = const.tile([128, B, HPC], BF16, tag="Q2")
        nc.vector.memset(Q2_sb[:], 0.0)
        nc.vector.tensor_copy(Q2_sb[0:64, :, 0], QT_sb[0:64, :])
        nc.vector.tensor_copy(Q2_sb[64:128, :, 1], QT_sb[64:128, :])

        # ---- main loop over batches ----
        kpool = ctx.enter_context(tc.tile_pool(name="kp", bufs=KV_BUFS))
        vpool = ctx.enter_context(tc.tile_pool(name="vp", bufs=KV_BUFS))
        epool = ctx.enter_context(tc.tile_pool(name="ep", bufs=3))
        spool = ctx.enter_context(tc.tile_pool(name="sp", bufs=2, space="PSUM"))

        xpsum = psum.tile([128, B, HPC], F32, tag="px")

        prev = None  # (b, vt, e) pending V-matmuls (software pipelining)

        def emit_v(pb, pvt, pe):
            for j in range(NJ):
                nc.tensor.matmul(
                    xpsum[:, pb, :], pvt[:, j], pe[:, j, :],
                    start=(j == 0), stop=(j == NJ - 1),
                )

        for b in [bb for _ in range(repeat) for bb in range(B)]:
            kt = kpool.tile([128, T], BF16, tag="k")
            vt = vpool.tile([128, NJ, HPC * DK], BF16, tag="v")
            nc.sync.dma_start(kt[:], kc[b])
            nc.sync.dma_start(vt[:], vcf[b].rearrange("p (j f) -> p j f", j=NJ))

            if variant == "dma":
                scr0 = epool.tile([128, NJ, HPC], BF16, tag="e")
                nc.vector.tensor_copy(scr0[:, 0, :], kt[:, 0:2])
                nc.vector.tensor_copy(scr0[:, 1, :], vt[:, 0, 0:2])
                continue

            # scores: 32 chunk matmuls -> sp [128=t, (j, h)]
            sp = spool.tile([128, NJ, HPC], F32, tag="s")
            for j in range(NJ):
                nc.tensor.matmul(
                    sp[:, j, :], kt[:, j * 128:(j + 1) * 128], Q2_sb[:, b, :],
                    start=True, stop=True,
                )

            # exp (scale 1/sqrt(dk)) + per-head denominator partials
            e = epool.tile([128, NJ, HPC], BF16, tag="e")
            nc.scalar.activation(
                e[:, :, 0], sp[:, :, 0], ACTF.Exp, scale=0.125,
                accum_out=dpartA[:, b:b + 1],
            )
            nc.scalar.activation(
                e[:, :, 1], sp[:, :, 1], ACTF.Exp, scale=0.125,
                accum_out=dpartB[:, b:b + 1],
            )

            if variant == "nope":
                continue

            if prev is not None:
                emit_v(*prev)
            prev = (b, vt, e)

        if variant == "full" and prev is not None:
            emit_v(*prev)

        # ---- epilogue ----
        small = ctx.enter_context(tc.tile_pool(name="small", bufs=1))

        if variant != "full":
            junk = small.tile([128, NCH * B], F32, tag="out")
            nc.vector.tensor_copy(junk[:], wq_sb[:, 0, :].unsqueeze(1).broadcast_to([128, 2, 128]))
            nc.sync.dma_start(outT, junk[:])

        if variant == "full":
            # x diagonal extraction: xn [128=(h,d), B] fp32
            xn = small.tile([128, B], F32, tag="xn")
            nc.vector.tensor_copy(xn[0:64, :], xpsum[0:64, :, 0])
            nc.vector.tensor_copy(xn[64:128, :], xpsum[64:128, :, 1])

            # new-token scores: s_new[h, b] = sum_d Q^T[.,b] * Knew^T[.,b]
            prod2 = small.tile([128, B], F32, tag="prod2")
            nc.vector.tensor_mul(prod2[:], QT_sb[:], KnT_sb[:])
            snpA = psum.tile([1, B], F32, tag="p0")
            snpB = psum.tile([1, B], F32, tag="p1")
            nc.tensor.matmul(snpA[0:1, :], ones_sb[0:64, 0:1], prod2[0:64, :],
                             start=True, stop=True, tile_position=(0, 0))
            nc.tensor.matmul(snpB[0:1, :], ones_sb[64:128, 0:1], prod2[64:128, :],
                             start=True, stop=True, tile_position=(64, 0))
            e_new = small.tile([1, 2 * B], F32, tag="enew")
            nc.scalar.activation(e_new[0:1, 0:B], snpA[0:1, :], ACTF.Exp, scale=0.125)
            nc.scalar.activation(e_new[0:1, B:2 * B], snpB[0:1, :], ACTF.Exp, scale=0.125)

            # broadcast e_new to [128, B] (rows by head half), fold v_new into x
            erp = spool.tile([128, B], F32, tag="s")
            nc.tensor.matmul(erp[0:64, :], onerow_sb[0:1, 0:64], e_new[0:1, 0:B],
                             start=True, stop=True, tile_position=(0, 0))
            nc.tensor.matmul(erp[64:128, :], onerow_sb[0:1, 0:64], e_new[0:1, B:2 * B],
                             start=True, stop=True, tile_position=(0, 64))
            tmp = small.tile([128, B], F32, tag="tmp")
            nc.vector.tensor_mul(tmp[:], VnT_sb[:], erp[:])
            xu = small.tile([128, B], F32, tag="xu")
            nc.vector.tensor_add(xu[:], tmp[:], xn[:])

            # denominators: full-partition sums of dpartA/dpartB + e_new
            dnpA = psum.tile([1, B], F32, tag="p2")
            dnpB = psum.tile([1, B], F32, tag="p3")
            nc.tensor.matmul(dnpA[0:1, :], ones_sb[:, 0:1], dpartA[:],
                             start=True, stop=True)
            nc.tensor.matmul(dnpB[0:1, :], ones_sb[:, 0:1], dpartB[:],
                             start=True, stop=True)
            dtot = small.tile([1, 2 * B], F32, tag="dtot")
            nc.vector.tensor_add(dtot[0:1, 0:B], dnpA[0:1, :], e_new[0:1, 0:B])
            nc.vector.tensor_add(dtot[0:1, B:2 * B], dnpB[0:1, :], e_new[0:1, B:2 * B])
            rcp = small.tile([1, 2 * B], F32, tag="rcp")
            nc.vector.reciprocal(rcp[0:1, :], dtot[0:1, :])

            rcpp = spool.tile([128, B], F32, tag="s")
            nc.tensor.matmul(rcpp[0:64, :], onerow_sb[0:1, 0:64], rcp[0:1, 0:B],
                             start=True, stop=True, tile_position=(0, 0))
            nc.tensor.matmul(rcpp[64:128, :], onerow_sb[0:1, 0:64], rcp[0:1, B:2 * B],
                             start=True, stop=True, tile_position=(0, 64))
            xs = small.tile([128, B], F32, tag="xs")
            nc.vector.tensor_mul(xs[:], xu[:], rcpp[:])

            # output projection: out^T chunks [128, B] = woT-chunk.T @ x (+ bo/8).
            # spool ping-pong keeps PE-writes off the bank ACT is reading.
            outsb = small.tile([128, NCH * B], F32, tag="out")
            for m in range(NCH):
                op = spool.tile([128, B], F32, tag="s")
                nc.tensor.matmul(op[:], wo_sb[:, m * 128:(m + 1) * 128], xs[:],
                                 start=True, stop=True)
                nc.scalar.activation(outsb[:, m * B:(m + 1) * B], op[:],
                                     ACTF.Identity, bias=cst_sb[:, 3 + m:4 + m], scale=1.0)
            nc.sync.dma_start(outT, outsb[:])

    nc.compile()
    return nc


_NC_CACHE = None


def _get_nc():
    global _NC_CACHE
    if _NC_CACHE is None:
        _NC_CACHE = _build_nc()
    return _NC_CACHE


def make_in_maps(q, key_pre, value_pre, wq, bq, wk, bk, wv, bv, wo, bo):
    bf = ml_dtypes.bfloat16
    q = np.asarray(q, np.float32)
    key_pre = np.asarray(key_pre, np.float32)
    value_pre = np.asarray(value_pre, np.float32)
    wq, bq = np.asarray(wq, np.float32), np.asarray(bq, np.float32)
    wk, bk = np.asarray(wk, np.float32), np.asarray(bk, np.float32)
    wv, bv = np.asarray(wv, np.float32), np.asarray(bv, np.float32)
    wo, bo = np.asarray(wo, np.float32), np.asarray(bo, np.float32)

    q2 = q.reshape(B, DM)
    qT8 = np.ascontiguousarray(q2.T.reshape(NCH, 128, B).transpose(1, 0, 2))
    bo8 = (bo / NCORES).reshape(NCH, 128).T  # [128, 8]

    in_maps = []
    for c in range(NCORES):
        hs = slice(c * HD, (c + 1) * HD)
        heads = slice(c * HPC, (c + 1) * HPC)
        cstv = np.zeros((128, 11), np.float32)
        cstv[:, 0] = bq[hs]
        cstv[:, 1] = bk[hs]
        cstv[:, 2] = bv[hs]
        cstv[:, 3:11] = bo8
        # K^T: [B, 2, T, DK] -> [B, 2, DK, T] -> [B, 128, T]
        kh = key_pre[:, heads].transpose(0, 1, 3, 2).reshape(B, HD, T)
        # V: [B, 2, T, DK] -> [B, 2, NJ, 128, DK] -> [B, 128, NJ, 2, DK]
        vh = value_pre[:, heads].reshape(B, HPC, NJ, 128, DK).transpose(0, 3, 2, 1, 4)
        in_maps.append({
            "qT8": qT8,
            "wq8": np.ascontiguousarray(wq[hs].T.reshape(NCH, 128, HD).transpose(1, 0, 2)),
            "wk8": np.ascontiguousarray(wk[hs].T.reshape(NCH, 128, HD).transpose(1, 0, 2)),
            "wv8": np.ascontiguousarray(wv[hs].T.reshape(NCH, 128, HD).transpose(1, 0, 2)),
            "woT": np.ascontiguousarray(wo[:, hs].T),
            "cst": cstv,
            "kc": np.ascontiguousarray(kh.astype(bf)),
            "vc": np.ascontiguousarray(vh.astype(bf)),
        })
    return in_maps


def gather_output(results):
    total = np.zeros((B, DM), np.float64)
    for c in range(NCORES):
        r = results[c]["outT"]  # [128, NCH*B]
        x = r.reshape(128, NCH, B).transpose(2, 1, 0).reshape(B, DM)
        total += x
    return total.astype(np.float32).reshape(B, 1, DM)


def run(in_maps, trace=False, **kw):
    nc = _get_nc()
    return run_bass_kernel_spmd(nc, in_maps, core_ids=list(range(NCORES)),
                                trace=trace, **kw)


def kernel(q, key_pre, value_pre, wq, bq, wk, bk, wv, bv, wo, bo):
    in_maps = make_in_maps(q, key_pre, value_pre, wq, bq, wk, bk, wv, bv, wo, bo)
    res = run(in_maps, trace=False)
    return gather_output(res.results)


# revision 7
# speedup vs baseline: 1.0763x; 1.0763x over previous
"""Trainium2 Bass kernel: caching self multi-headed attention (decode step).

Problem: B=32, QLEN=1, DM=1024, H=16, DK=64, TCACHE=4096, fp32 in/out.
  out = MHA(q; KV cache) with QKV projections, cache append, softmax, out-proj.

Sharding (8 NeuronCores): tensor-parallel over heads. Core c owns heads
[2c, 2c+1]: column-parallel wq/wk/wv (128 output dims per core), KV cache
shards on the head dim, row-parallel wo giving a partial [32, 1024] output
per core; the host sums the 8 partials.

v2 design (HW-bound analysis of v1: DMA 88% @ ~314 GB/s AND PE 91% busy with
4096 fp32 LDW+MM pairs; DVE 61% with 1x-rate tensor_reduce):
  * KV cache cast to bf16 on the host -> HBM traffic halves (69 MB/core).
    Host-simulated end-to-end rel err 2.9e-3 (gate 2e-2).
  * All attention math moves to PE as 128-contraction bf16 matmuls:
      scores: per t-chunk j (32 of 128 t), lhsT = K^T chunk [128=(h,d), 128=t]
        (stationary, bf16 fast-weight-load), rhs = Q2 [128, 2] block-column
        (col h holds Q_h on head h's 64 rows, 0 elsewhere)
        -> out [128=t, 2=h] in PSUM, all 32 chunks -> [128, (j,h)] one bank.
      exp: 2 strided ACT ops (one per head) -> e [128,(j,h)] bf16 with
        per-head per-partition denominator accumulation (fp32).
      x:  per t-chunk j, lhsT = V chunk [128=t, 128=(h,d)] (stationary FWL),
        rhs = e[:, j, :] [128, 2] -> out [128=(h,d), 2] accumulated over j in
        PSUM; diagonal (head-matching) half used, off-head column is waste.
    PE cost ~64 small LDW+MM pairs/batch (~2us) -- under the DMA floor.
  * DVE does nothing in the main loop. K rides the sync HWDGE ring, V the
    scalar ring.
  * No per-batch SWDGE q-broadcasts (v1 had 64): Q2 block-columns are built
    once from the projection output with 2 strided copies.

Per-core HBM floor: 69 MB @ ~320-358 GB/s => ~190-215 us.

Softmax skips the max-subtraction: scores ~ N(0,1), exp is safe in fp32 and
mathematically identical to the reference.
"""

import numpy as np
import ml_dtypes
from contextlib import ExitStack

import concourse.bass as bass
import concourse.tile as tile
from concourse import bacc, mybir
from concourse.bass_utils import run_bass_kernel_spmd

F32 = mybir.dt.float32
BF16 = mybir.dt.bfloat16
AX = mybir.AxisListType
ALU = mybir.AluOpType
ACTF = mybir.ActivationFunctionType

B = 32          # batch
DM = 1024       # model dim
H = 16          # total heads
DK = 64         # head dim
T = 4096        # cache length
NCORES = 8
HPC = H // NCORES   # 2 heads per core
HD = HPC * DK       # 128 per-core head dims
NCH = DM // 128     # 8 contraction chunks for the projections
NJ = T // 128       # 32 t-chunks of 128

KV_BUFS = 7         # merged KV tile prefetch depth (16 KB/partition)


def _build_nc(repeat=1, variant="full"):
    # variant: "full" | "dma" (K/V loads only) | "nope" (no V matmuls)
    nc = bacc.Bacc(
        "TRN2",
        target_bir_lowering=False,
        debug=False,
        enable_asserts=False,
        num_devices=NCORES,
    )

    qT8 = nc.dram_tensor("qT8", [128, NCH, B], BF16, kind="ExternalInput").ap()
    wq8 = nc.dram_tensor("wq8", [128, NCH, HD], BF16, kind="ExternalInput").ap()
    wk8 = nc.dram_tensor("wk8", [128, NCH, HD], BF16, kind="ExternalInput").ap()
    wv8 = nc.dram_tensor("wv8", [128, NCH, HD], BF16, kind="ExternalInput").ap()
    woT = nc.dram_tensor("woT", [HD, DM], F32, kind="ExternalInput").ap()
    cst = nc.dram_tensor("cst", [128, 11], F32, kind="ExternalInput").ap()
    # merged KV: [b, 128, 0:T]=K^T [(h,d), t] ; [b, 128, T:2T]=V [tloc, (j,h,d)]
    kvc = nc.dram_tensor("kvc", [B, 128, 2 * T], BF16, kind="ExternalInput").ap()
    outT = nc.dram_tensor("outT", [128, NCH * B], F32, kind="ExternalOutput").ap()

    with ExitStack() as ctx:
        tc = ctx.enter_context(tile.TileContext(nc))
        const = ctx.enter_context(tc.tile_pool(name="const", bufs=1))
        psum = ctx.enter_context(tc.tile_pool(name="psum", bufs=1, space="PSUM"))

        # ---- constants into SBUF ----
        wq_sb = const.tile([128, NCH, HD], BF16, tag="wq")
        wk_sb = const.tile([128, NCH, HD], BF16, tag="wk")
        wv_sb = const.tile([128, NCH, HD], BF16, tag="wv")
        wo_sb = const.tile([HD, DM], F32, tag="wo")
        qT_sb = const.tile([128, NCH, B], BF16, tag="qt")
        cst_sb = const.tile([128, 11], F32, tag="cst")
        # weights ride the scalar HWDGE ring so the sync ring starts on the
        # KV stream at t=0 (KV is the critical 69 MB; weights are ~2 MB)
        nc.scalar.dma_start(wq_sb[:], wq8)
        nc.scalar.dma_start(wk_sb[:], wk8)
        nc.scalar.dma_start(wv_sb[:], wv8)
        nc.scalar.dma_start(wo_sb[:], woT)
        nc.scalar.dma_start(qT_sb[:], qT8)
        nc.scalar.dma_start(cst_sb[:], cst)

        ones_sb = const.tile([128, 1], F32, tag="ones")
        onerow_sb = const.tile([1, 64], F32, tag="onerow")
        nc.vector.memset(ones_sb[:], 1.0)
        nc.vector.memset(onerow_sb[:], 1.0)

        dpartA = const.tile([128, B], F32, tag="dpA")  # head-0 denom partials
        dpartB = const.tile([128, B], F32, tag="dpB")  # head-1 denom partials

        # ---- phase 0: projections Q^T, Knew^T, Vnew^T  [128=(h,d), B] ----
        QTp = psum.tile([128, B], F32, tag="p0")
        KTp = psum.tile([128, B], F32, tag="p1")
        VTp = psum.tile([128, B], F32, tag="p2")
        for c in range(NCH):
            st, sp = (c == 0), (c == NCH - 1)
            nc.tensor.matmul(QTp[:], wq_sb[:, c, :], qT_sb[:, c, :], start=st, stop=sp)
        for c in range(NCH):
            st, sp = (c == 0), (c == NCH - 1)
            nc.tensor.matmul(KTp[:], wk_sb[:, c, :], qT_sb[:, c, :], start=st, stop=sp)
        for c in range(NCH):
            st, sp = (c == 0), (c == NCH - 1)
            nc.tensor.matmul(VTp[:], wv_sb[:, c, :], qT_sb[:, c, :], start=st, stop=sp)

        QT_sb = const.tile([128, B], F32, tag="QT")
        KnT_sb = const.tile([128, B], F32, tag="KnT")
        VnT_sb = const.tile([128, B], F32, tag="VnT")
        nc.scalar.activation(QT_sb[:], QTp[:], ACTF.Identity, bias=cst_sb[:, 0:1], scale=1.0)
        nc.scalar.activation(KnT_sb[:], KTp[:], ACTF.Identity, bias=cst_sb[:, 1:2], scale=1.0)
        nc.scalar.activation(VnT_sb[:], VTp[:], ACTF.Identity, bias=cst_sb[:, 2:3], scale=1.0)

        # Q2 block-columns [128, B, 2] bf16: col (b,h) = Q_h masked to head h rows
        Q2_sb = const.tile([128, B, HPC], BF16, tag="Q2")
        nc.vector.memset(Q2_sb[:], 0.0)
        nc.vector.tensor_copy(Q2_sb[0:64, :, 0], QT_sb[0:64, :])
        nc.vector.tensor_copy(Q2_sb[64:128, :, 1], QT_sb[64:128, :])

        # ---- main loop over batches ----
        kpool = ctx.enter_context(tc.tile_pool(name="kp", bufs=KV_BUFS))
        epool = ctx.enter_context(tc.tile_pool(name="ep", bufs=3))
        spool = ctx.enter_context(tc.tile_pool(name="sp", bufs=2, space="PSUM"))

        xpsum = psum.tile([128, B, HPC], F32, tag="px")

        prev = None  # (b, vt, e) pending V-matmuls (software pipelining)

        def emit_v(pb, pvt, pe):
            for j in range(NJ):
                nc.tensor.matmul(
                    xpsum[:, pb, :], pvt[:, j], pe[:, j, :],
                    start=(j == 0), stop=(j == NJ - 1),
                )

        for b in [bb for _ in range(repeat) for bb in range(B)]:
            kvt = kpool.tile([128, 2 * T], BF16, tag="kv")
            nc.sync.dma_start(kvt[:], kvc[b])
            kt = kvt[:, 0:T]
            vt = kvt[:, T:2 * T].rearrange("p (j f) -> p j f", j=NJ)

            if variant == "dma":
                scr0 = epool.tile([128, NJ, HPC], BF16, tag="e")
                nc.vector.tensor_copy(scr0[:, 0, :], kt[:, 0:2])
                nc.vector.tensor_copy(scr0[:, 1, :], vt[:, 0, 0:2])
                continue

            # scores: 32 chunk matmuls -> sp [128=t, (j, h)]
            sp = spool.tile([128, NJ, HPC], F32, tag="s")
            for j in range(NJ):
                nc.tensor.matmul(
                    sp[:, j, :], kt[:, j * 128:(j + 1) * 128], Q2_sb[:, b, :],
                    start=True, stop=True,
                )

            # exp (scale 1/sqrt(dk)) + per-head denominator partials
            e = epool.tile([128, NJ, HPC], BF16, tag="e")
            nc.scalar.activation(
                e[:, :, 0], sp[:, :, 0], ACTF.Exp, scale=0.125,
                accum_out=dpartA[:, b:b + 1],
            )
            nc.scalar.activation(
                e[:, :, 1], sp[:, :, 1], ACTF.Exp, scale=0.125,
                accum_out=dpartB[:, b:b + 1],
            )

            if variant == "nope":
                continue

            if prev is not None:
                emit_v(*prev)
            prev = (b, vt, e)

        if variant == "full" and prev is not None:
            emit_v(*prev)

        # ---- epilogue ----
        small = ctx.enter_context(tc.tile_pool(name="small", bufs=1))

        if variant != "full":
            junk = small.tile([128, NCH * B], F32, tag="out")
            nc.vector.tensor_copy(junk[:], wq_sb[:, 0, :].unsqueeze(1).broadcast_to([128, 2, 128]))
            nc.sync.dma_start(outT, junk[:])

        if variant == "full":
            # x diagonal extraction: xn [128=(h,d), B] fp32
            xn = small.tile([128, B], F32, tag="xn")
            nc.vector.tensor_copy(xn[0:64, :], xpsum[0:64, :, 0])
            nc.vector.tensor_copy(xn[64:128, :], xpsum[64:128, :, 1])

            # new-token scores: s_new[h, b] = sum_d Q^T[.,b] * Knew^T[.,b]
            prod2 = small.tile([128, B], F32, tag="prod2")
            nc.vector.tensor_mul(prod2[:], QT_sb[:], KnT_sb[:])
            snpA = psum.tile([1, B], F32, tag="p0")
            snpB = psum.tile([1, B], F32, tag="p1")
            nc.tensor.matmul(snpA[0:1, :], ones_sb[0:64, 0:1], prod2[0:64, :],
                             start=True, stop=True, tile_position=(0, 0))
            nc.tensor.matmul(snpB[0:1, :], ones_sb[64:128, 0:1], prod2[64:128, :],
                             start=True, stop=True, tile_position=(64, 0))
            e_new = small.tile([1, 2 * B], F32, tag="enew")
            nc.scalar.activation(e_new[0:1, 0:B], snpA[0:1, :], ACTF.Exp, scale=0.125)
            nc.scalar.activation(e_new[0:1, B:2 * B], snpB[0:1, :], ACTF.Exp, scale=0.125)

            # broadcast e_new to [128, B] (rows by head half), fold v_new into x
            erp = spool.tile([128, B], F32, tag="s")
            nc.tensor.matmul(erp[0:64, :], onerow_sb[0:1, 0:64], e_new[0:1, 0:B],
                             start=True, stop=True, tile_position=(0, 0))
            nc.tensor.matmul(erp[64:128, :], onerow_sb[0:1, 0:64], e_new[0:1, B:2 * B],
                             start=True, stop=True, tile_position=(0, 64))
            tmp = small.tile([128, B], F32, tag="tmp")
            nc.vector.tensor_mul(tmp[:], VnT_sb[:], erp[:])
            xu = small.tile([128, B], F32, tag="xu")
            nc.vector.tensor_add(xu[:], tmp[:], xn[:])

            # denominators: full-partition sums of dpartA/dpartB + e_new
            dnpA = psum.tile([1, B], F32, tag="p2")
            dnpB = psum.tile([1, B], F32, tag="p3")
            nc.tensor.matmul(dnpA[0:1, :], ones_sb[:, 0:1], dpartA[:],
                             start=True, stop=True)
            nc.tensor.matmul(dnpB[0:1, :], ones_sb[:, 0:1], dpartB[:],
                             start=True, stop=True)
            dtot = small.tile([1, 2 * B], F32, tag="dtot")
            nc.vector.tensor_add(dtot[0:1, 0:B], dnpA[0:1, :], e_new[0:1, 0:B])
            nc.vector.tensor_add(dtot[0:1, B:2 * B], dnpB[0:1, :], e_new[0:1, B:2 * B])
            rcp = small.tile([1, 2 * B], F32, tag="rcp")
            nc.vector.reciprocal(rcp[0:1, :], dtot[0:1, :])

            rcpp = spool.tile([128, B], F32, tag="s")
            nc.tensor.matmul(rcpp[0:64, :], onerow_sb[0:1, 0:64], rcp[0:1, 0:B],
                             start=True, stop=True, tile_position=(0, 0))
            nc.tensor.matmul(rcpp[64:128, :], onerow_sb[0:1, 0:64], rcp[0:1, B:2 * B],
                             start=True, stop=True, tile_position=(0, 64))
            xs = small.tile([128, B], F32, tag="xs")
            nc.vector.tensor_mul(xs[:], xu[:], rcpp[:])

            # output projection: out^T chunks [128, B] = woT-chunk.T @ x (+ bo/8).
            # spool ping-pong keeps PE-writes off the bank ACT is reading.
            outsb = small.tile([128, NCH * B], F32, tag="out")
            for m in range(NCH):
                op = spool.tile([128, B], F32, tag="s")
                nc.tensor.matmul(op[:], wo_sb[:, m * 128:(m + 1) * 128], xs[:],
                                 start=True, stop=True)
                nc.scalar.activation(outsb[:, m * B:(m + 1) * B], op[:],
                                     ACTF.Identity, bias=cst_sb[:, 3 + m:4 + m], scale=1.0)
            nc.sync.dma_start(outT, outsb[:])

    nc.compile()
    return nc


_NC_CACHE = None


def _get_nc():
    global _NC_CACHE
    if _NC_CACHE is None:
        _NC_CACHE = _build_nc()
    return _NC_CACHE


def make_in_maps(q, key_pre, value_pre, wq, bq, wk, bk, wv, bv, wo, bo):
    bf = ml_dtypes.bfloat16
    q = np.asarray(q, np.float32)
    key_pre = np.asarray(key_pre, np.float32)
    value_pre = np.asarray(value_pre, np.float32)
    wq, bq = np.asarray(wq, np.float32), np.asarray(bq, np.float32)
    wk, bk = np.asarray(wk, np.float32), np.asarray(bk, np.float32)
    wv, bv = np.asarray(wv, np.float32), np.asarray(bv, np.float32)
    wo, bo = np.asarray(wo, np.float32), np.asarray(bo, np.float32)

    q2 = q.reshape(B, DM)
    qT8 = np.ascontiguousarray(q2.T.reshape(NCH, 128, B).transpose(1, 0, 2))
    bo8 = (bo / NCORES).reshape(NCH, 128).T  # [128, 8]

    in_maps = []
    for c in range(NCORES):
        hs = slice(c * HD, (c + 1) * HD)
        heads = slice(c * HPC, (c + 1) * HPC)
        cstv = np.zeros((128, 11), np.float32)
        cstv[:, 0] = bq[hs]
        cstv[:, 1] = bk[hs]
        cstv[:, 2] = bv[hs]
        cstv[:, 3:11] = bo8
        # K^T: [B, 2, T, DK] -> [B, 2, DK, T] -> [B, 128, T]
        kh = key_pre[:, heads].transpose(0, 1, 3, 2).reshape(B, HD, T).astype(bf)
        # V: [B, 2, T, DK] -> [B, 2, NJ, 128, DK] -> [B, 128, NJ, 2, DK]
        vh = value_pre[:, heads].reshape(B, HPC, NJ, 128, DK) \
            .transpose(0, 3, 2, 1, 4).reshape(B, 128, T).astype(bf)
        kv = np.concatenate([kh, vh], axis=2)
        in_maps.append({
            "qT8": qT8.astype(bf),
            "wq8": np.ascontiguousarray(wq[hs].T.reshape(NCH, 128, HD).transpose(1, 0, 2)).astype(bf),
            "wk8": np.ascontiguousarray(wk[hs].T.reshape(NCH, 128, HD).transpose(1, 0, 2)).astype(bf),
            "wv8": np.ascontiguousarray(wv[hs].T.reshape(NCH, 128, HD).transpose(1, 0, 2)).astype(bf),
            "woT": np.ascontiguousarray(wo[:, hs].T),
            "cst": cstv,
            "kvc": np.ascontiguousarray(kv),
        })
    return in_maps


def gather_output(results):
    total = np.zeros((B, DM), np.float64)
    for c in range(NCORES):
        r = results[c]["outT"]  # [128, NCH*B]
        x = r.reshape(128, NCH, B).transpose(2, 1, 0).reshape(B, DM)
        total += x
    return total.astype(np.float32).reshape(B, 1, DM)


def run(in_maps, trace=False, **kw):
    nc = _get_nc()
    return run_bass_kernel_spmd(nc, in_maps, core_ids=list(range(NCORES)),
                                trace=trace, **kw)


def kernel(q, key_pre, value_pre, wq, bq, wk, bk, wv, bv, wo, bo):
    in_maps = make_in_maps(q, key_pre, value_pre, wq, bq, wk, bk, wv, bv, wo, bo)
    res = run(in_maps, trace=False)
    return gather_output(res.results)
